# revision 2
# baseline (speedup 1.0000x reference)
"""Trainium2 Bass kernel for nn_CoverageLoss (retrieval_knn).

Math reduction: the loss only needs, per space sample s, the 4 smallest L1
distances to all latents.  Each core computes, for its latent shard
[N/8, 64], the 8 smallest distances per space sample; the host merges the
8x8 candidates and finishes the tiny reduction (tail means -> top-64 rows
-> Huber mean).

Device algorithm (thermometer-matmul): with a uniform grid t_k = -1 + k*d,
d = 2/K over [-1, 1] (space samples always lie inside), encode
  u_k(a) = clamp((a - t_k)/d, 0, 1)          (soft code, exact)
  v_k(b) = 1[round((clip(b) + 1)/d) > k]     (hard code, b quantized)
Then sum_k d*|u_k - v_k| == |a - bq| exactly (one side binary), so
  L1(a_s, b_n) = Arow(s) + Bcol(n) - 2d * (U_s . V_n)
with Arow = sum_d (a+1), Bcol = sum_d (bq+1) + overflow(|b|>1) both exact on
host.  The whole [S, N] distance matrix becomes ONE bf16 matmul with
contraction C = 64*(K+1) (slots padded to K+1 per dim; two spare slots
carry -(Bcol - Bmean)/(2d) so PSUM holds P with argmax P == argmin L1).
The DVE only runs Max8 directly on PSUM chunks; the only approximation is
b's grid rounding (rel loss err ~7e-3 measured for K=7, gate 2e-2).
"""

import numpy as np
import ml_dtypes
from contextlib import ExitStack

S = 2048
N = 65536
D = 64
NCORES = 8
NLOC = N // NCORES  # 8192
K = 7               # soft levels per dim
SL = K + 1          # slots per dim -> C = D*SL = 512
C = D * SL
NCI = C // 128      # 4 contraction chunks
LO = -1.0
DELTA = 2.0 / K
CHUNK = 512         # matmul moving free dim / PSUM bank columns
GRP = 4             # psum banks per group (2 groups ping-pong)

_cache = {}


def _build(nloc=NLOC, s=S):
    import concourse.tile as tile
    from concourse import bacc, mybir

    nc = bacc.Bacc(
        "TRN2",
        target_bir_lowering=False,
        debug=False,
        num_devices=NCORES,
    )
    f32 = mybir.dt.float32
    bf16 = mybir.dt.bfloat16

    a_enc = nc.dram_tensor("aEnc", [128, NCI * s], bf16, kind="ExternalInput").ap()
    b_enc = nc.dram_tensor("bEnc", [128, NCI * nloc], bf16, kind="ExternalInput").ap()
    tails = nc.dram_tensor("tails", [s, 8], f32, kind="ExternalOutput").ap()

    n_sblocks = s // 128
    n_chunks = nloc // CHUNK          # 16
    n_grps = n_chunks // GRP          # 4

    with tile.TileContext(nc) as tc, ExitStack() as ctx:
        const_pool = ctx.enter_context(tc.tile_pool(name="const", bufs=1))
        psum_pool = ctx.enter_context(
            tc.tile_pool(name="psum", bufs=8, space="PSUM")
        )
        out_pool = ctx.enter_context(tc.tile_pool(name="outs", bufs=2))

        # Stationary codes for all space samples (small, load first).
        asb = const_pool.tile([128, NCI * s], bf16)
        nc.sync.dma_start(asb[:, :], a_enc[:, :])

        # Latent codes, DMA'd grp-major so the first group's columns (all 4
        # ci chunks) land first and the PE can start within ~10us.
        bsb = const_pool.tile([128, NCI * nloc], bf16)
        for g in range(n_grps):
            cols = slice(g * GRP * CHUNK, (g + 1) * GRP * CHUNK)
            for ci in range(NCI):
                nc.sync.dma_start(
                    bsb[:, ci * nloc + g * GRP * CHUNK: ci * nloc + (g + 1) * GRP * CHUNK],
                    b_enc[:, ci * nloc + g * GRP * CHUNK: ci * nloc + (g + 1) * GRP * CHUNK],
                )

        # Warm the PE (HAM clock gate) while the first input DMAs land.
        dummy = const_pool.tile([128, CHUNK], bf16)
        nc.vector.memset(dummy[:, :], 0.0)
        warm = psum_pool.tile([128, CHUNK], f32, space="PSUM", tag="psumb", name="psumb")
        for _ in range(40):
            nc.tensor.matmul(
                warm[:, :], dummy[:, 0:128], dummy[:, :],
                start=True, stop=True,
            )

        for sb in range(n_sblocks):
            cand = out_pool.tile([128, n_chunks * 8], f32, name="cand")
            for g in range(n_grps):
                psums = [
                    psum_pool.tile([128, CHUNK], f32, space="PSUM", tag="psumb", name="psumb")
                    for _ in range(GRP)
                ]
                for ci in range(NCI):
                    lhs = asb[:, ci * s + sb * 128: ci * s + (sb + 1) * 128]
                    for j in range(GRP):
                        ncol = (g * GRP + j) * CHUNK
                        nc.tensor.matmul(
                            psums[j][:, :],
                            lhs,
                            bsb[:, ci * nloc + ncol: ci * nloc + ncol + CHUNK],
                            start=(ci == 0),
                            stop=(ci == NCI - 1),
                        )
                for j in range(GRP):
                    nc.vector.max(
                        out=cand[:, (g * GRP + j) * 8: (g * GRP + j) * 8 + 8],
                        in_=psums[j][:, :],
                    )
            top = out_pool.tile([128, 8], f32, name="top")
            nc.vector.max(out=top[:, :], in_=cand[:, :])
            nc.sync.dma_start(tails[sb * 128: (sb + 1) * 128, :], top[:, :])

    nc.compile()
    return nc


def _get_nc(nloc=NLOC, s=S):
    key = (nloc, s)
    if key not in _cache:
        _cache[key] = _build(nloc, s)
    return _cache[key]


def _encode(latents, ss):
    """Host-side thermometer codes.  Returns per-core input maps + finish data."""
    lat = np.asarray(latents, dtype=np.float32)
    ss = np.asarray(ss, dtype=np.float32)
    s, d = ss.shape
    n = lat.shape[0]

    # hard code for latents (b), with exact overflow correction
    bc = np.clip(lat, LO, LO + K * DELTA)
    m = np.round((bc - LO) / DELTA)                    # [N, D] in [0, K]
    bq = LO + m * DELTA
    ov = np.abs(lat - bc).sum(axis=1)                  # [N]
    bcol = (bq - LO).sum(axis=1) + ov                  # [N]
    bmean = np.float32(bcol.mean())

    ks = np.arange(SL, dtype=np.float32)
    v = (m[:, :, None] > ks[None, None, :]).astype(np.float32)  # [N, D, SL]
    v[:, :, K:] = 0.0
    x = -(bcol - bmean) / (2.0 * DELTA)
    hi = np.round(x)
    lo_r = (x - hi).astype(ml_dtypes.bfloat16).astype(np.float32)
    v[:, 0, SL - 1] = hi
    v[:, 1, SL - 1] = lo_r
    v = v.reshape(n, C).astype(ml_dtypes.bfloat16)

    # soft code for space samples (a) -- exact
    t = LO + ks * DELTA
    u = np.clip((ss[:, :, None] - t[None, None, :]) / DELTA, 0.0, 1.0)
    u[:, :, K:] = 0.0
    u[:, 0, SL - 1] = 1.0
    u[:, 1, SL - 1] = 1.0
    u = u.reshape(s, C).astype(ml_dtypes.bfloat16)

    arow = (ss - LO).sum(axis=1).astype(np.float32)    # [S]

    # device layouts: [128 partitions = C rows of chunk ci, ci-major columns]
    a_dram = np.ascontiguousarray(
        u.T.reshape(NCI, 128, s).transpose(1, 0, 2).reshape(128, NCI * s)
    )
    in_maps = []
    for c in range(NCORES):
        vc = v[c * NLOC: (c + 1) * NLOC]               # [nloc, C]
        b_dram = np.ascontiguousarray(
            vc.T.reshape(NCI, 128, NLOC).transpose(1, 0, 2).reshape(128, NCI * NLOC)
        )
        in_maps.append({"aEnc": a_dram, "bEnc": b_dram})
    return in_maps, arow, bmean


def _finish(per_core_tails, arow, bmean):
    """per_core_tails: [ncores, S, 8] Max8 of P; d = arow + bmean - 2*DELTA*P."""
    p = np.concatenate(list(per_core_tails), axis=1).reshape(arow.shape[0], -1)
    d = arow[:, None] + bmean - 2.0 * DELTA * p        # [S, 64]
    d.sort(axis=1)
    tail = d[:, :4]
    tail_mean = tail.mean(axis=1)
    far = np.argsort(-tail_mean, kind="stable")[:64]
    close = d[far][:, :4]
    a = np.abs(close)
    huber = np.where(a <= 1.0, 0.5 * close * close, a - 0.5)
    return np.float32(huber.mean())


def _run_device(latents, space_samples, trace=False):
    from concourse.bass_utils import run_bass_kernel_spmd

    nc = _get_nc()
    in_maps, arow, bmean = _encode(latents, space_samples)
    res = run_bass_kernel_spmd(nc, in_maps, list(range(NCORES)), trace=trace)
    tails = np.stack([res.results[c]["tails"] for c in range(NCORES)])
    return tails, arow, bmean, res


def kernel(latents, space_samples):
    tails, arow, bmean, _ = _run_device(latents, space_samples, trace=False)
    return _finish(tails, arow, bmean)


def run_traced(latents, space_samples):
    """Like kernel() but with NTFF profiling; returns (loss, exec_time_ns)."""
    tails, arow, bmean, res = _run_device(latents, space_samples, trace=True)
    return _finish(tails, arow, bmean), res.exec_time_ns


# revision 3
# speedup vs baseline: 1.4111x; 1.4111x over previous
"""Trainium2 Bass kernel for nn_CoverageLoss (retrieval_knn).

Math reduction: the loss only needs, per space sample s, the 4 smallest L1
distances to all latents.  Each core computes, for its latent shard
[N/8, 64], the 8 smallest distances per space sample; the host merges the
8x8 candidates and finishes the tiny reduction (tail means -> top-64 rows
-> Huber mean).

Device algorithm (thermometer-matmul): with a uniform grid t_k = -1 + k*d,
d = 2/K over [-1, 1] (space samples always lie inside), encode
  u_k(a) = clamp((a - t_k)/d, 0, 1)          (soft code, exact)
  v_k(b) = 1[round((clip(b) + 1)/d) > k]     (hard code, b quantized)
Then sum_k d*|u_k - v_k| == |a - bq| exactly (one side binary), so
  L1(a_s, b_n) = Arow(s) + Bcol(n) - 2d * (U_s . V_n)
with Arow = sum_d (a+1), Bcol = sum_d (bq+1) + overflow(|b|>1) both exact on
host.  The whole [S, N] distance matrix becomes ONE fp8 matmul with
contraction C = 64*(K+1); three spare slots carry a hi2/hi/lo split of
-(Bcol - Bmean)/(2d) (each piece exactly representable in fp8e4m3) so PSUM
holds P with argmax P == argmin L1.  Matmuls run DoubleRow (2 fp8
contraction rows per pass).  The scalar engine evicts PSUM - rowref(s) to
bf16 SBUF (rowref recenters so candidate values sit near 0, keeping bf16
rounding ~0.04 in distance units), and the DVE runs Max8 on the bf16
chunks.  Only approximations: b's grid rounding + bf16 eviction rounding
(rel loss err ~7.3e-3 measured for K=7 against the gate of 2e-2).
"""

import numpy as np
import ml_dtypes
from contextlib import ExitStack

S = 2048
N = 65536
D = 64
NCORES = 8
NLOC = N // NCORES  # 8192
K = 7               # soft levels per dim
SL = K + 1          # slots per dim -> C = D*SL = 512
C = D * SL
NCI = C // 128      # 4 contraction chunks
NPAIR = NCI // 2    # DoubleRow processes chunk pairs
LO = -1.0
DELTA = 2.0 / K
DCTR = 45.0         # recenter distances about this before bf16 eviction
CHUNK = 512         # matmul moving free dim / PSUM bank columns
GRP = 4             # psum banks per group (2 groups ping-pong)

_cache = {}


def _build(nloc=NLOC, s=S):
    import concourse.tile as tile
    from concourse import bacc, mybir

    nc = bacc.Bacc(
        "TRN2",
        target_bir_lowering=False,
        debug=False,
        num_devices=NCORES,
    )
    f32 = mybir.dt.float32
    bf16 = mybir.dt.bfloat16
    fp8 = mybir.dt.float8e4
    ident = mybir.ActivationFunctionType.Identity

    a_enc = nc.dram_tensor("aEnc", [128, NCI * s], fp8, kind="ExternalInput").ap()
    b_enc = nc.dram_tensor("bEnc", [128, NCI * nloc], fp8, kind="ExternalInput").ap()
    rref = nc.dram_tensor("rref", [128, s // 128], f32, kind="ExternalInput").ap()
    tails = nc.dram_tensor("tails", [s, 8], bf16, kind="ExternalOutput").ap()

    n_sblocks = s // 128
    n_chunks = nloc // CHUNK          # 16
    n_grps = n_chunks // GRP          # 4

    with tile.TileContext(nc) as tc, ExitStack() as ctx:
        const_pool = ctx.enter_context(tc.tile_pool(name="const", bufs=1))
        psum_pool = ctx.enter_context(
            tc.tile_pool(name="psum", bufs=8, space="PSUM")
        )
        evict_pool = ctx.enter_context(tc.tile_pool(name="evict", bufs=6))
        out_pool = ctx.enter_context(tc.tile_pool(name="outs", bufs=2))

        # Stationary codes for all space samples + row recenter consts.
        asb = const_pool.tile([128, NCI, s], fp8)
        nc.sync.dma_start(asb[:, :, :], a_enc[:, :])
        rsb = const_pool.tile([128, s // 128], f32)
        nc.sync.dma_start(rsb[:, :], rref[:, :])

        # Latent codes, DMA'd grp-major so the first group's columns (all
        # NCI chunks) land first and the PE can start within ~5us.
        bsb = const_pool.tile([128, NCI, nloc], fp8)
        for g in range(n_grps):
            for ci in range(NCI):
                nc.sync.dma_start(
                    bsb[:, ci, g * GRP * CHUNK: (g + 1) * GRP * CHUNK],
                    b_enc[:, ci * nloc + g * GRP * CHUNK: ci * nloc + (g + 1) * GRP * CHUNK],
                )

        # Warm the PE (HAM clock gate) while the first input DMAs land.
        dummy = const_pool.tile([128, CHUNK], bf16)
        nc.vector.memset(dummy[:, :], 0.0)
        warm = psum_pool.tile([128, CHUNK], f32, space="PSUM", tag="psumb", name="psumb")
        for _ in range(30):
            nc.tensor.matmul(
                warm[:, :], dummy[:, 0:128], dummy[:, :],
                start=True, stop=True,
            )

        for sb in range(n_sblocks):
            cand = out_pool.tile([128, n_chunks * 8], bf16, name="cand")
            for g in range(n_grps):
                psums = [
                    psum_pool.tile([128, CHUNK], f32, space="PSUM", tag="psumb", name="psumb")
                    for _ in range(GRP)
                ]
                for p in range(NPAIR):
                    lhs = asb[:, 2 * p: 2 * p + 2, sb * 128: (sb + 1) * 128]
                    for j in range(GRP):
                        ncol = (g * GRP + j) * CHUNK
                        nc.tensor.matmul(
                            psums[j][:, :],
                            lhs,
                            bsb[:, 2 * p: 2 * p + 2, ncol: ncol + CHUNK],
                            start=(p == 0),
                            stop=(p == NPAIR - 1),
                            perf_mode=mybir.MatmulPerfMode.DoubleRow,
                        )
                for j in range(GRP):
                    xb = evict_pool.tile([128, CHUNK], bf16, name="xb")
                    nc.scalar.activation(
                        xb[:, :], psums[j][:, :], ident,
                        bias=rsb[:, sb: sb + 1], scale=1.0,
                    )
                    nc.vector.max(
                        out=cand[:, (g * GRP + j) * 8: (g * GRP + j) * 8 + 8],
                        in_=xb[:, :],
                    )
            top = out_pool.tile([128, 8], bf16, name="top")
            nc.vector.max(out=top[:, :], in_=cand[:, :])
            nc.sync.dma_start(tails[sb * 128: (sb + 1) * 128, :], top[:, :])

    nc.compile()
    return nc


def _get_nc(nloc=NLOC, s=S):
    key = (nloc, s)
    if key not in _cache:
        _cache[key] = _build(nloc, s)
    return _cache[key]


def _encode(latents, ss):
    """Host-side thermometer codes.  Returns per-core input maps + finish data."""
    fp8 = ml_dtypes.float8_e4m3fn
    lat = np.asarray(latents, dtype=np.float32)
    ss = np.asarray(ss, dtype=np.float32)
    s, d = ss.shape
    n = lat.shape[0]

    # hard code for latents (b), with exact overflow correction
    bc = np.clip(lat, LO, LO + K * DELTA)
    m = np.round((bc - LO) / DELTA)                    # [N, D] in [0, K]
    bq = LO + m * DELTA
    ov = np.abs(lat - bc).sum(axis=1)                  # [N]
    bcol = (bq - LO).sum(axis=1) + ov                  # [N]
    bmean = np.float32(bcol.mean())

    ks = np.arange(SL, dtype=np.float32)
    v = (m[:, :, None] > ks[None, None, :]).astype(np.float32)  # [N, D, SL]
    v[:, :, K:] = 0.0
    # fold rows: hi2/hi/lo split of -(bcol-bmean)/(2d), each fp8-exact
    x = -(bcol - bmean) / (2.0 * DELTA)
    hi2 = np.round(x / 16.0) * 16.0
    r = x - hi2
    hi = np.round(r)
    lo_r = r - hi
    v[:, 0, SL - 1] = hi2
    v[:, 1, SL - 1] = hi
    v[:, 2, SL - 1] = lo_r
    v = v.reshape(n, C).astype(fp8)

    # soft code for space samples (a) -- exact
    t = LO + ks * DELTA
    u = np.clip((ss[:, :, None] - t[None, None, :]) / DELTA, 0.0, 1.0)
    u[:, :, K:] = 0.0
    u[:, 0, SL - 1] = 1.0
    u[:, 1, SL - 1] = 1.0
    u[:, 2, SL - 1] = 1.0
    u = u.reshape(s, C).astype(fp8)

    arow = (ss - LO).sum(axis=1).astype(np.float32)    # [S]
    rowref = ((arow + bmean - DCTR) / (2.0 * DELTA)).astype(np.float32)
    rref = np.ascontiguousarray(-rowref.reshape(s // 128, 128).T)  # [128, s//128]

    # device layouts: [128 partitions = C rows of chunk ci, ci-major columns]
    a_dram = np.ascontiguousarray(
        u.T.reshape(NCI, 128, s).transpose(1, 0, 2).reshape(128, NCI * s)
    )
    in_maps = []
    for c in range(NCORES):
        vc = v[c * NLOC: (c + 1) * NLOC]               # [nloc, C]
        b_dram = np.ascontiguousarray(
            vc.T.reshape(NCI, 128, NLOC).transpose(1, 0, 2).reshape(128, NCI * NLOC)
        )
        in_maps.append({"aEnc": a_dram, "bEnc": b_dram, "rref": rref})
    return in_maps, arow, bmean, rowref


def _finish(per_core_tails, arow, bmean, rowref):
    """per_core_tails: [ncores, S, 8] Max8 of (P - rowref) in bf16."""
    xs = np.concatenate(list(per_core_tails), axis=1).reshape(arow.shape[0], -1)
    xs = xs.astype(np.float32)
    d = (arow[:, None] + bmean) - 2.0 * DELTA * (xs + rowref[:, None])  # [S, 64]
    d.sort(axis=1)
    tail = d[:, :4]
    tail_mean = tail.mean(axis=1)
    far = np.argsort(-tail_mean, kind="stable")[:64]
    close = d[far][:, :4]
    a = np.abs(close)
    huber = np.where(a <= 1.0, 0.5 * close * close, a - 0.5)
    return np.float32(huber.mean())


def _run_device(latents, space_samples, trace=False):
    from concourse.bass_utils import run_bass_kernel_spmd

    nc = _get_nc()
    in_maps, arow, bmean, rowref = _encode(latents, space_samples)
    res = run_bass_kernel_spmd(nc, in_maps, list(range(NCORES)), trace=trace)
    tails = np.stack([res.results[c]["tails"] for c in range(NCORES)])
    return tails, arow, bmean, rowref, res


def kernel(latents, space_samples):
    tails, arow, bmean, rowref, _ = _run_device(latents, space_samples, trace=False)
    return _finish(tails, arow, bmean, rowref)


def run_traced(latents, space_samples):
    """Like kernel() but with NTFF profiling; returns (loss, exec_time_ns)."""
    tails, arow, bmean, rowref, res = _run_device(latents, space_samples, trace=True)
    return _finish(tails, arow, bmean, rowref), res.exec_time_ns
